# revision 28
# baseline (speedup 1.0000x reference)
"""DistinctionLoss Trainium2 kernel v3e (raw bacc, hand-scheduled).

Math (per batch b):
  f_n = x_n / ||x_n||;  s = sum_n f_n;  mean(gram) = ||s||^2 / N^2
  dot_n = rn_n * (x_n . s);  sim_n = (dot_n - 1)/(N-1)
  bce*N = -(sum max(ls,-100) - sum relu(sim)*w),  w = ls_c - l1_c
  loss = bce + 1 - mean_b(||s_b||^2)/N^2

Measured engine rates per [128,256] group (unthrottled -- HAM halves
all rates if the kernel runs past ~26us, so total span matters):
  DVE fused STT+accum ~420ns | ACT Square/Identity+accum ~775ns
  GPSIMD TensorTensor ~790ns | DVE slab tensor_reduce ~290ns/group
  PE matmul ~213ns effective

Two cores share each batch: the host uploads the SAME batch to cores
(2c, 2c+1) but with the 32 row-groups rotated by 16 for the odd core.
Row order is irrelevant to ssq/rn/s/||s||^2, so both cores compute the
same s; each computes pass2 dots only for its FIRST 16 groups (its
half of the rows), and the host combines lssum - rwsum_A - rwsum_B.
This halves pass2 with zero cross-core traffic and one shared NEFF.

Schedule:
  - pass1 (all 32 groups on every core): DVE fused STT majority, ACT
    Square+accum ~1/3, GPSIMD squares a few groups that DVE reduces as
    one slab tensor_reduce per chunk.
  - rn via rsqrt bit-trick (2 int tensor_scalar, no Newton): feeds both
    the PE weights and the dots. ~2% rn error -> <1e-5 on the loss.
  - ACT warms Ln first so one activation table set (natural_log:
    ln+square+identity+copy) serves the whole kernel: 1 table load.
  - PE: rn-stationary matmuls accumulate s per chunk; broadcast via
    ones-matmul.
  - pass2 (16 groups): DVE fused STT for 11, GPSIMD products + ACT
    Identity+accum for 5.
  - outputs per core: [sum max(ls,-100), rwsum_half, ||s||^2].
"""

import os

import numpy as np
import ml_dtypes

B = 8
N, D, P = 4096, 256, 128
G = N // P
CH = [4, 7, 7, 7, 5, 2]
NCH = len(CH)
OFF = [sum(CH[:i]) for i in range(NCH)]
NINV = 1.0 / (N - 1)
LOG_CLAMP = -100.0
N_WARM = int(os.environ.get("V3_WARM", "16"))

# pass1 per-chunk split: (dve_fused, act_fused)
P1_SPLIT = [(4, 0), (5, 2), (4, 3), (4, 3), (2, 3), (1, 1)]
assert [sum(t) for t in P1_SPLIT] == CH, (P1_SPLIT, CH)

# pass2: all 32 groups; first P2_DVE fused on DVE, rest GP->ACT lane
P2_N = G
P2_DVE = int(os.environ.get("V3_P2_DVE", "21"))   # DVE-fused; rest GP->ACT

MAGIC = 0x5F3759DF

_cache = {}


def _build_nc():
    import concourse.bacc as bacc
    from concourse import mybir
    from contextlib import ExitStack

    fp32 = mybir.dt.float32
    bf16 = mybir.dt.bfloat16
    i32 = mybir.dt.int32
    AF = mybir.ActivationFunctionType
    ALU = mybir.AluOpType
    AX = mybir.AxisListType

    nc = bacc.Bacc(
        "TRN2", target_bir_lowering=False, debug=False,
        enable_asserts=False, num_devices=8,
    )

    xd = nc.dram_tensor("xbf", [P, G * D], bf16, kind="ExternalInput")
    scd = nc.dram_tensor("scores", [P, G], fp32, kind="ExternalInput")
    out_d = nc.dram_tensor("out", [1, 3], fp32, kind="ExternalOutput")

    NGP = P2_N - P2_DVE

    sb = nc.alloc_sbuf_tensor
    x_t = sb("x", [P, G, D], bf16)
    prod_t = sb("prod", [P, G, D], bf16)     # GP squares / pass2 products
    ptv_t = sb("ptv", [P, D], bf16)          # DVE fused product sink
    sqa_t = sb("sqa", [P, D], bf16)          # ACT square sink
    red_t = sb("red", [P, D], fp32)          # ACT identity-reduce sink
    pts_t = sb("pts", [1, D], fp32)          # ||s||^2 product sink
    ssq_t = sb("ssq", [P, G], fp32)
    y0_t = sb("y0", [P, G], fp32)            # bit-trick rsqrt (used as rn)
    t1_t = sb("t1", [P, G], fp32)            # scratch
    rnbf_t = sb("rnbf", [P, G], bf16)        # PE stationary weights
    sc_t = sb("sc", [P, G], fp32)
    ls_t = sb("ls", [P, G], fp32)
    l1_t = sb("l1", [P, G], fp32)
    w_t = sb("w", [P, G], fp32)
    draw_t = sb("draw", [P, P2_N], fp32)
    dots_t = sb("dots", [P, P2_N], fp32)
    sim_t = sb("sim", [P, P2_N], fp32)
    rterm_t = sb("rterm", [P, P2_N], fp32)
    onesb_t = sb("onesb", [1, P], bf16)
    onesf_t = sb("onesf", [P, 1], fp32)
    sbf1_t = sb("sbf1", [1, D], bf16)
    sbc_t = sb("sbc", [P, D], bf16)
    outsb_t = sb("outsb", [P, 3], fp32)
    outfin_t = sb("outfin", [1, 3], fp32)
    warm_t = sb("warm", [1, 2], fp32)

    ctx = ExitStack()
    ps_s = ctx.enter_context(nc.psum_tensor([1, D], fp32))
    ps_bc = ctx.enter_context(nc.psum_tensor([P, D], fp32))
    ps_tot = ctx.enter_context(nc.psum_tensor([1, 3], fp32))

    names = ([f"S_dx{k}" for k in range(NCH)] +
             [f"S_p1a{k}" for k in range(NCH)] +
             [f"S_y0{k}" for k in range(NCH)] +
             ["S_dsc", "S_ln", "S_rn", "S_pe", "S_sbf", "S_pebc",
              "S_sbc", "S_prodA", "S_acc", "S_dve", "S_pef",
              "S_fin", "S_ones", "S_od"])
    S = {n: ctx.enter_context(nc.semaphore(n)) for n in names}
    S_dx = [S[f"S_dx{k}"] for k in range(NCH)]
    S_p1a = [S[f"S_p1a{k}"] for k in range(NCH)]
    S_y0 = [S[f"S_y0{k}"] for k in range(NCH)]

    def gsl(k):
        return slice(OFF[k], OFF[k] + CH[k])

    def p1_ranges(k):
        d, a = P1_SPLIT[k]
        o = OFF[k]
        return range(o, o + d), range(o + d, o + d + a)

    with ctx, nc.Block() as block:
        @block.sync
        def _(sync):
            for k in range(NCH):
                sync.dma_start(
                    out=x_t[:, gsl(k), :],
                    in_=xd[:, OFF[k] * D:(OFF[k] + CH[k]) * D],
                ).then_inc(S_dx[k], 16)
            sync.wait_ge(S["S_fin"], 1)
            sync.dma_start(out=out_d[:], in_=outfin_t[:]).then_inc(S["S_od"], 16)
            sync.wait_ge(S["S_od"], 16)

        @block.scalar
        def _(act):
            # warm with Ln so the chosen table set (natural_log) also
            # holds square/identity/copy -> one table load total
            act.activation(out=warm_t[:, 0:1],
                           in_=nc.const_aps.tensor(1.0, (1, 1)), func=AF.Ln)
            act.activation(out=warm_t[:, 1:2],
                           in_=nc.const_aps.tensor(1.0, (1, 1)), func=AF.Square)
            for k in range(NCH):
                _, ra = p1_ranges(k)
                if len(ra) == 0:
                    continue
                act.wait_ge(S_dx[k], 16)
                mm = None
                for g in ra:
                    mm = act.activation(
                        out=sqa_t[:], in_=x_t[:, g, :], func=AF.Square,
                        accum_out=ssq_t[:, g:g + 1],
                    )
                mm.then_inc(S_p1a[k], 1)
            act.wait_ge(S["S_dsc"], 16)
            act.activation(out=ls_t[:], in_=sc_t[:], func=AF.Ln)
            act.activation(out=l1_t[:], in_=sc_t[:], func=AF.Ln,
                           scale=-1.0, bias=1.0).then_inc(S["S_ln"], 1)
            # pass2 ACT lane: reduce GP's product groups via Identity+accum
            mm = None
            for i in range(NGP):
                g = P2_DVE + i
                act.wait_ge(S["S_prodA"], i + 1)
                mm = act.activation(
                    out=red_t[:], in_=prod_t[:, g, :], func=AF.Identity,
                    accum_out=draw_t[:, g:g + 1],
                )
            mm.then_inc(S["S_acc"], 1)
            act.wait_ge(S["S_pef"], 1)
            act.copy(outfin_t[:], ps_tot[:]).then_inc(S["S_fin"], 1)

        @block.vector
        def _(dve):
            dve.memset(onesb_t[:], 1.0)
            dve.memset(onesf_t[:], 1.0)
            dve.memset(outsb_t[:], 0.0).then_inc(S["S_ones"], 1)

            def magic(k):
                # rn for chunk k: rsqrt bit trick on the fp32 ssq.
                # Runs one chunk late (except the last), so the accum
                # writes have retired and the drains are near-free.
                if P1_SPLIT[k][1]:
                    dve.wait_ge(S_p1a[k], 1)
                dve.drain()
                cs = gsl(k)
                dve.tensor_scalar(
                    out=t1_t[:, cs].bitcast(i32), in0=ssq_t[:, cs].bitcast(i32),
                    scalar1=1, scalar2=-1,
                    op0=ALU.logical_shift_right, op1=ALU.bitwise_xor,
                )
                dve.drain()
                dve.tensor_scalar(
                    out=y0_t[:, cs].bitcast(i32), in0=t1_t[:, cs].bitcast(i32),
                    scalar1=MAGIC + 1, scalar2=None, op0=ALU.add,
                ).then_inc(S_y0[k], 1)

            for k in range(NCH):
                rd, ra = p1_ranges(k)
                dve.wait_ge(S_dx[k], 16)
                if k > 0:
                    magic(k - 1)
                for g in rd:
                    dve.scalar_tensor_tensor(
                        out=ptv_t[:], in0=x_t[:, g, :], scalar=0.0,
                        in1=x_t[:, g, :], op0=ALU.bypass, op1=ALU.mult,
                        accum_out=ssq_t[:, g:g + 1],
                    )
            magic(NCH - 1)
            # s arrives: copy [1,D] psum -> sbuf bf16, broadcast via PE
            dve.wait_ge(S["S_pe"], 1)
            dve.tensor_copy(out=sbf1_t[:], in_=ps_s[:]).then_inc(S["S_sbf"], 1)
            dve.wait_ge(S["S_pebc"], 1)
            dve.drain()
            dve.tensor_copy(out=sbc_t[:], in_=ps_bc[:]).then_inc(S["S_sbc"], 1)
            dve.drain()
            # ||s||^2 off the critical path
            dve.scalar_tensor_tensor(
                out=pts_t[:], in0=sbf1_t[:], scalar=0.0, in1=sbf1_t[:],
                op0=ALU.bypass, op1=ALU.mult, accum_out=outsb_t[0:1, 2:3],
            )
            # pass2 DVE lane: fused dots for groups 0..P2_DVE-1
            for g in range(P2_DVE):
                dve.scalar_tensor_tensor(
                    out=ptv_t[:], in0=x_t[:, g, :], scalar=0.0,
                    in1=sbc_t[:], op0=ALU.bypass, op1=ALU.mult,
                    accum_out=draw_t[:, g:g + 1],
                )
            # scores tail: clamp logs, w, lssum -> outsb col0
            dve.wait_ge(S["S_ln"], 1)
            dve.tensor_scalar_max(ls_t[:], ls_t[:], LOG_CLAMP)
            dve.tensor_scalar_max(l1_t[:], l1_t[:], LOG_CLAMP)
            dve.drain()
            dve.tensor_sub(w_t[:], ls_t[:], l1_t[:])
            dve.tensor_reduce(out=outsb_t[:, 0:1], in_=ls_t[:], axis=AX.X,
                              op=ALU.add)
            dve.wait_ge(S["S_acc"], 1)
            dve.drain()
            dve.tensor_mul(dots_t[:], draw_t[:], y0_t[:, 0:P2_N])
            dve.drain()
            dve.tensor_scalar(
                out=sim_t[:], in0=dots_t[:], scalar1=1.0, scalar2=NINV,
                op0=ALU.subtract, op1=ALU.mult,
            )
            dve.drain()
            dve.scalar_tensor_tensor(
                out=rterm_t[:], in0=sim_t[:], scalar=0.0, in1=w_t[:, 0:P2_N],
                op0=ALU.max, op1=ALU.mult, accum_out=outsb_t[:, 1:2],
            ).then_inc(S["S_dve"], 1)

        @block.gpsimd
        def _(gp):
            gp.dma_start(out=sc_t[:], in_=scd[:]).then_inc(S["S_dsc"], 16)
            for k in range(NCH):
                gp.wait_ge(S_y0[k], 1)
                gp.tensor_copy(out=rnbf_t[:, gsl(k)], in_=y0_t[:, gsl(k)]
                               ).then_inc(S["S_rn"], 1)
            # pass2 products for the ACT reduce lane
            gp.wait_ge(S["S_sbc"], 1)
            for i in range(NGP):
                g = P2_DVE + i
                gp.tensor_mul(prod_t[:, g, :], x_t[:, g, :], sbc_t[:]
                              ).then_inc(S["S_prodA"], 1)

        @block.tensor
        def _(pe):
            pe.wait_ge(S["S_ones"], 1)
            for _ in range(N_WARM):
                pe.matmul(ps_bc[:, 0:P], onesb_t[:], onesb_t[:],
                          start=True, stop=True)
            mm = None
            for k in range(NCH):
                pe.wait_ge(S["S_rn"], k + 1)
                for g in range(OFF[k], OFF[k] + CH[k]):
                    mm = pe.matmul(
                        ps_s[:], rnbf_t[:, g:g + 1], x_t[:, g, :],
                        start=(g == 0), stop=(g == G - 1),
                    )
            mm.then_inc(S["S_pe"], 1)
            pe.wait_ge(S["S_sbf"], 1)
            pe.matmul(ps_bc[:], onesb_t[:], sbf1_t[:], start=True, stop=True
                      ).then_inc(S["S_pebc"], 1)
            pe.wait_ge(S["S_dve"], 1)
            pe.matmul(ps_tot[:], onesf_t[:], outsb_t[:], start=True, stop=True
                      ).then_inc(S["S_pef"], 1)

    nc.finalize()
    return nc


def _get_nc():
    if "nc" not in _cache:
        _cache["nc"] = _build_nc()
    return _cache["nc"]


def run_on_device(features: np.ndarray, scores: np.ndarray, trace: bool = False,
                  tmpdir: str | None = None):
    """Returns (per_core_outputs [8, 3] float64, BassKernelResults).

    Per-core out: [sum max(ls,-100), sum relu(sim)*w, ||s||^2].
    """
    from concourse.bass_utils import run_bass_kernel_spmd

    nc = _get_nc()
    in_maps = []
    for c in range(B):
        in_maps.append({
            "xbf": np.ascontiguousarray(features[c]).reshape(P, G * D)
            .astype(ml_dtypes.bfloat16),
            "scores": np.ascontiguousarray(scores[c]).reshape(P, G)
            .astype(np.float32),
        })
    res = run_bass_kernel_spmd(nc, in_maps, core_ids=list(range(B)),
                               trace=trace, tmpdir=tmpdir)
    outs = np.stack([res.results[c]["out"].reshape(3) for c in range(B)])
    return outs.astype(np.float64), res


def combine(outs: np.ndarray) -> np.float32:
    """Host-side reduction of the 8 per-core [ls_sum, rwsum, ssq_s]."""
    bce = np.mean(-(outs[:, 0] - outs[:, 1]) / N)
    feat = 1.0 - np.sum(outs[:, 2]) / (B * float(N) * float(N))
    return np.float32(bce + feat)


def kernel(features: np.ndarray, scores: np.ndarray) -> np.ndarray:
    outs, _ = run_on_device(features, scores)
    return np.asarray(combine(outs), dtype=np.float32)


# revision 29
# speedup vs baseline: 1.1766x; 1.1766x over previous
"""DistinctionLoss Trainium2 kernel v2m (v2 schedule + bit-trick rsqrt).

Math (per batch b, one batch per core):
  f_n = x_n / ||x_n||                       (row-normalized features)
  s   = sum_n f_n                           ([D] weighted row sum)
  mean(gram) = ||s||^2 / N^2                (the N x N gram is never built)
  dot_n = f_n . s = rn_n * (x_n . s)
  sim_n = (dot_n - 1)/(N-1);  t_n = 1 - relu(sim_n)
  bce  = -mean(t*log(sc) + (1-t)*log1p(-sc))   (logs clamped at -100)
       = -mean(ls - relu(sim)*w),  w = ls - l1
  loss = bce + 1 - mean_b(||s_b||^2)/N^2

v2m vs v2:
  - rn = 1/||x_n|| via the fp32 rsqrt bit trick (two int tensor_scalar
    ops on DVE, no Newton; ~2% rn error perturbs the loss by <1e-5
    because the feature-dependent terms are ~1e-4 of the total) with a
    GPSIMD cast to bf16 for the PE weights. This removes the per-chunk
    DVE reciprocal + ACT sqrt chain -- and with it the 6 ACT table
    reloads (1.28us each) that sat on the rn -> PE critical path.
  - ACT warms Ln first so the single activation table set that holds
    ln+square+identity+copy serves the whole kernel (1 load total).
"""

import os

import numpy as np
import ml_dtypes

B = 8
N, D, P = 4096, 256, 128
G = N // P
CH = [8, 8, 8, 6, 2]
NCH = len(CH)
OFF = [sum(CH[:i]) for i in range(NCH)]
ACT_SQ = [2, 2, 2, 2, 0]  # trailing groups per chunk squared on ACT
NINV = 1.0 / (N - 1)
LOG_CLAMP = -100.0
N_WARM = int(os.environ.get("V2_WARM", "24"))
MAGIC = 0x5F3759DF

_cache = {}


def _build_nc():
    import concourse.bacc as bacc
    from concourse import mybir
    from contextlib import ExitStack

    fp32 = mybir.dt.float32
    bf16 = mybir.dt.bfloat16
    i32 = mybir.dt.int32
    AF = mybir.ActivationFunctionType
    ALU = mybir.AluOpType
    AX = mybir.AxisListType

    nc = bacc.Bacc(
        "TRN2", target_bir_lowering=False, debug=False,
        enable_asserts=False, num_devices=8,
    )

    xd = nc.dram_tensor("xbf", [P, G * D], bf16, kind="ExternalInput")
    scd = nc.dram_tensor("scores", [P, G], fp32, kind="ExternalInput")
    out_d = nc.dram_tensor("out", [1, 2], fp32, kind="ExternalOutput")

    sb = nc.alloc_sbuf_tensor
    x_t = sb("x", [P, G, D], bf16)
    pt_t = sb("pt", [P, G, D], bf16)     # DVE TTR product sink (per group)
    sqa_t = sb("sqa", [P, 8, D], bf16)   # ACT square sinks (per square)
    ssq_t = sb("ssq", [P, G], fp32)
    t1_t = sb("t1", [P, G], fp32)        # magic scratch
    y0_t = sb("y0", [P, G], fp32)        # bit-trick rsqrt
    rnbf_t = sb("rnbf", [P, G], bf16)
    sc_t = sb("sc", [P, G], fp32)
    ls_t = sb("ls", [P, G], fp32)
    l1_t = sb("l1", [P, G], fp32)
    w_t = sb("w", [P, G], fp32)
    lssum_t = sb("lssum", [P, 1], fp32)
    draw_t = sb("draw", [P, G], fp32)
    dots_t = sb("dots", [P, G], fp32)
    sim_t = sb("sim", [P, G], fp32)
    rterm_t = sb("rterm", [P, G], fp32)
    rwsum_t = sb("rwsum", [P, 1], fp32)
    onesb_t = sb("onesb", [1, P], bf16)
    onesf_t = sb("onesf", [P, 1], fp32)
    sbf1_t = sb("sbf1", [1, D], bf16)
    sbc_t = sb("sbc", [P, D], bf16)
    outsb_t = sb("outsb", [P, 2], fp32)
    outfin_t = sb("outfin", [1, 2], fp32)
    warm_t = sb("warm", [1, 2], fp32)

    ctx = ExitStack()
    ps_s = ctx.enter_context(nc.psum_tensor([1, D], fp32))
    ps_bc = ctx.enter_context(nc.psum_tensor([P, D], fp32))
    ps_tot = ctx.enter_context(nc.psum_tensor([1, 2], fp32))
    names = ([f"S_dx{k}" for k in range(NCH)] +
             [f"S_y0{k}" for k in range(NCH)] +
             ["S_dsc", "S_ln", "S_sqa", "S_rnbf", "S_pe",
              "S_sbf", "S_pebc", "S_sbc", "S_dve", "S_pef", "S_fin",
              "S_ones", "S_od"])
    S = {n: ctx.enter_context(nc.semaphore(n)) for n in names}
    S_dx = [S[f"S_dx{k}"] for k in range(NCH)]
    S_y0 = [S[f"S_y0{k}"] for k in range(NCH)]

    def gsl(k):
        return slice(OFF[k], OFF[k] + CH[k])

    with ctx, nc.Block() as block:
        @block.sync
        def _(sync):
            for k in range(NCH):
                sync.dma_start(
                    out=x_t[:, gsl(k), :],
                    in_=xd[:, OFF[k] * D:(OFF[k] + CH[k]) * D],
                ).then_inc(S_dx[k], 16)
            sync.wait_ge(S["S_fin"], 1)
            sync.dma_start(out=out_d[:], in_=outfin_t[:]).then_inc(S["S_od"], 16)
            sync.wait_ge(S["S_od"], 16)

        @block.gpsimd
        def _(gp):
            gp.dma_start(out=sc_t[:], in_=scd[:]).then_inc(S["S_dsc"], 16)
            # cast the bit-trick rsqrt to bf16 PE weights, per chunk
            for k in range(NCH):
                gp.wait_ge(S_y0[k], 1)
                gp.tensor_copy(out=rnbf_t[:, gsl(k)], in_=y0_t[:, gsl(k)]
                               ).then_inc(S["S_rnbf"], 1)

        @block.scalar
        def _(act):
            # warm Ln first: the natural_log table set also holds
            # square/identity/copy, so this is the only table load
            act.activation(out=warm_t[:, 0:1],
                           in_=nc.const_aps.tensor(1.0, (1, 1)), func=AF.Ln)
            act.activation(out=warm_t[:, 1:2],
                           in_=nc.const_aps.tensor(1.0, (1, 1)), func=AF.Square)
            for k in range(NCH):
                na = ACT_SQ[k]
                if na:
                    act.wait_ge(S_dx[k], 16)
                    mm = None
                    for j in range(na):
                        g = OFF[k] + CH[k] - na + j
                        mm = act.activation(
                            out=sqa_t[:, 2 * k + j, :], in_=x_t[:, g, :],
                            func=AF.Square,
                            accum_out=ssq_t[:, g:g + 1],
                        )
                    mm.then_inc(S["S_sqa"], 1)
                if k == 0:
                    # scores logs ride behind chunk 0's work
                    act.wait_ge(S["S_dsc"], 16)
                    act.activation(out=ls_t[:], in_=sc_t[:], func=AF.Ln)
                    act.activation(out=l1_t[:], in_=sc_t[:], func=AF.Ln,
                                   scale=-1.0, bias=1.0).then_inc(S["S_ln"], 1)
            act.wait_ge(S["S_pe"], 1)
            act.copy(sbf1_t[:], ps_s[:]).then_inc(S["S_sbf"], 1)
            act.wait_ge(S["S_pebc"], 1)
            act.copy(sbc_t[:], ps_bc[:]).then_inc(S["S_sbc"], 1)
            act.wait_ge(S["S_pef"], 1)
            act.copy(outfin_t[:], ps_tot[:]).then_inc(S["S_fin"], 1)

        @block.vector
        def _(dve):
            dve.memset(onesb_t[:], 1.0)
            dve.memset(onesf_t[:], 1.0)
            dve.memset(outsb_t[:], 0.0).then_inc(S["S_ones"], 1)
            nsq = 0
            for k in range(NCH):
                dve.wait_ge(S_dx[k], 16)
                for g in range(OFF[k], OFF[k] + CH[k] - ACT_SQ[k]):
                    dve.scalar_tensor_tensor(
                        out=pt_t[:, g, :], in0=x_t[:, g, :], scalar=0.0,
                        in1=x_t[:, g, :], op0=ALU.bypass, op1=ALU.mult,
                        accum_out=ssq_t[:, g:g + 1],
                    )
                if ACT_SQ[k]:
                    nsq += 1
                    dve.wait_ge(S["S_sqa"], nsq)
                dve.drain()
                # rn for this chunk: rsqrt bit trick on the fp32 ssq
                cs = gsl(k)
                dve.tensor_scalar(
                    out=t1_t[:, cs].bitcast(i32), in0=ssq_t[:, cs].bitcast(i32),
                    scalar1=1, scalar2=-1,
                    op0=ALU.logical_shift_right, op1=ALU.bitwise_xor,
                )
                dve.drain()
                dve.tensor_scalar(
                    out=y0_t[:, cs].bitcast(i32), in0=t1_t[:, cs].bitcast(i32),
                    scalar1=MAGIC + 1, scalar2=None, op0=ALU.add,
                ).then_inc(S_y0[k], 1)
                if k == 1:
                    # scores tail rides behind chunk 1 (S_ln long since up)
                    dve.wait_ge(S["S_ln"], 1)
                    dve.tensor_scalar_max(ls_t[:], ls_t[:], LOG_CLAMP)
                    dve.tensor_scalar_max(l1_t[:], l1_t[:], LOG_CLAMP)
                    dve.drain()
                    dve.tensor_sub(w_t[:], ls_t[:], l1_t[:])
                    dve.tensor_reduce(out=lssum_t[:], in_=ls_t[:], axis=AX.X,
                                      op=ALU.add)
            # ||s||^2 from the bf16 SBUF copy of s (error ~1e-9 on the loss)
            dve.wait_ge(S["S_sbf"], 1)
            dve.scalar_tensor_tensor(
                out=pt_t[0:1, 0, :], in0=sbf1_t[:], scalar=0.0,
                in1=sbf1_t[:], op0=ALU.bypass, op1=ALU.mult,
                accum_out=outsb_t[0:1, 1:2],
            )
            dve.drain()
            # pass2: per-row dot with broadcast s
            dve.wait_ge(S["S_sbc"], 1)
            for g in range(G):
                dve.scalar_tensor_tensor(
                    out=pt_t[:, g, :], in0=x_t[:, g, :], scalar=0.0,
                    in1=sbc_t[:], op0=ALU.bypass, op1=ALU.mult,
                    accum_out=draw_t[:, g:g + 1],
                )
            dve.drain()
            dve.tensor_mul(dots_t[:], draw_t[:], y0_t[:])
            dve.drain()
            dve.tensor_scalar(
                out=sim_t[:], in0=dots_t[:], scalar1=1.0, scalar2=NINV,
                op0=ALU.subtract, op1=ALU.mult,
            )
            dve.drain()
            dve.scalar_tensor_tensor(
                out=rterm_t[:], in0=sim_t[:], scalar=0.0, in1=w_t[:],
                op0=ALU.max, op1=ALU.mult, accum_out=rwsum_t[:],
            )
            dve.drain()
            dve.tensor_sub(outsb_t[:, 0:1], lssum_t[:], rwsum_t[:]
                           ).then_inc(S["S_dve"], 1)

        @block.tensor
        def _(pe):
            # keep PE busy from the start so HAM unthrottles before the
            # real accumulation matmuls
            pe.wait_ge(S["S_ones"], 1)
            for _ in range(N_WARM):
                pe.matmul(ps_bc[:, 0:P], onesb_t[:], onesb_t[:],
                          start=True, stop=True)
            mm = None
            for k in range(NCH):
                pe.wait_ge(S["S_rnbf"], k + 1)
                for g in range(OFF[k], OFF[k] + CH[k]):
                    mm = pe.matmul(
                        ps_s[:], rnbf_t[:, g:g + 1], x_t[:, g, :],
                        start=(g == 0), stop=(g == G - 1),
                    )
            mm.then_inc(S["S_pe"], 1)
            pe.wait_ge(S["S_sbf"], 1)
            pe.matmul(ps_bc[:], onesb_t[:], sbf1_t[:], start=True, stop=True
                      ).then_inc(S["S_pebc"], 1)
            pe.wait_ge(S["S_dve"], 1)
            pe.matmul(ps_tot[:], onesf_t[:], outsb_t[:], start=True, stop=True
                      ).then_inc(S["S_pef"], 1)

    nc.finalize()
    return nc


def _get_nc():
    if "nc" not in _cache:
        _cache["nc"] = _build_nc()
    return _cache["nc"]


def run_on_device(features: np.ndarray, scores: np.ndarray, trace: bool = False,
                  tmpdir: str | None = None):
    """Returns (per_core_outputs [8, 2] float64, BassKernelResults)."""
    from concourse.bass_utils import run_bass_kernel_spmd

    nc = _get_nc()
    in_maps = []
    for c in range(B):
        in_maps.append({
            "xbf": np.ascontiguousarray(features[c]).reshape(P, G * D)
            .astype(ml_dtypes.bfloat16),
            "scores": np.ascontiguousarray(scores[c]).reshape(P, G)
            .astype(np.float32),
        })
    res = run_bass_kernel_spmd(nc, in_maps, core_ids=list(range(B)),
                               trace=trace, tmpdir=tmpdir)
    outs = np.stack([res.results[c]["out"].reshape(2) for c in range(B)])
    return outs.astype(np.float64), res


def combine(outs: np.ndarray) -> np.float32:
    """Host-side reduction of the 8 per-core [bce_sum, ssq_s] pairs."""
    bce = np.mean(-outs[:, 0] / N)
    feat = 1.0 - np.sum(outs[:, 1]) / (B * float(N) * float(N))
    return np.float32(bce + feat)


def kernel(features: np.ndarray, scores: np.ndarray) -> np.ndarray:
    outs, _ = run_on_device(features, scores)
    return np.asarray(combine(outs), dtype=np.float32)


# revision 41
# speedup vs baseline: 1.1803x; 1.0031x over previous
"""DistinctionLoss Trainium2 kernel v2m (v2 schedule + bit-trick rsqrt).

Math (per batch b, one batch per core):
  f_n = x_n / ||x_n||                       (row-normalized features)
  s   = sum_n f_n                           ([D] weighted row sum)
  mean(gram) = ||s||^2 / N^2                (the N x N gram is never built)
  dot_n = f_n . s = rn_n * (x_n . s)
  sim_n = (dot_n - 1)/(N-1);  t_n = 1 - relu(sim_n)
  bce  = -mean(t*log(sc) + (1-t)*log1p(-sc))   (logs clamped at -100)
       = -mean(ls - relu(sim)*w),  w = ls - l1
  loss = bce + 1 - mean_b(||s_b||^2)/N^2

v2m vs v2:
  - rn = 1/||x_n|| via the fp32 rsqrt bit trick (two int tensor_scalar
    ops on DVE, no Newton; ~2% rn error perturbs the loss by <1e-5
    because the feature-dependent terms are ~1e-4 of the total) with a
    GPSIMD cast to bf16 for the PE weights. This removes the per-chunk
    DVE reciprocal + ACT sqrt chain -- and with it the 6 ACT table
    reloads (1.28us each) that sat on the rn -> PE critical path.
  - ACT warms Ln first so the single activation table set that holds
    ln+square+identity+copy serves the whole kernel (1 load total).
"""

import os

import numpy as np
import ml_dtypes

B = 8
N, D, P = 4096, 256, 128
G = N // P
CH = [4, 8, 8, 8, 4]
NCH = len(CH)
OFF = [sum(CH[:i]) for i in range(NCH)]
ACT_SQ = [1, 3, 3, 2, 1]  # trailing groups per chunk squared on ACT
NINV = 1.0 / (N - 1)
LOG_CLAMP = -100.0
N_WARM = int(os.environ.get("V2_WARM", "24"))
MAGIC = 0x5F3759DF

_cache = {}


def _build_nc():
    import concourse.bacc as bacc
    from concourse import mybir
    from contextlib import ExitStack

    fp32 = mybir.dt.float32
    bf16 = mybir.dt.bfloat16
    i32 = mybir.dt.int32
    AF = mybir.ActivationFunctionType
    ALU = mybir.AluOpType
    AX = mybir.AxisListType

    nc = bacc.Bacc(
        "TRN2", target_bir_lowering=False, debug=False,
        enable_asserts=False, num_devices=8,
    )

    xd = nc.dram_tensor("xbf", [P, G * D], bf16, kind="ExternalInput")
    scd = nc.dram_tensor("scores", [P, G], fp32, kind="ExternalInput")
    out_d = nc.dram_tensor("out", [P, 2], fp32, kind="ExternalOutput")

    sb = nc.alloc_sbuf_tensor
    x_t = sb("x", [P, G, D], bf16)
    pt_t = sb("pt", [P, G, D], bf16)     # DVE TTR product sink (per group)
    sqa_t = sb("sqa", [P, 12, D], bf16)  # ACT square sinks (per square)
    ssq_t = sb("ssq", [P, G], fp32)
    t1_t = sb("t1", [P, G], fp32)        # magic scratch
    y0_t = sb("y0", [P, G], fp32)        # bit-trick rsqrt
    rnbf_t = sb("rnbf", [P, G], bf16)
    sc_t = sb("sc", [P, G], fp32)
    ls_t = sb("ls", [P, G], fp32)
    l1_t = sb("l1", [P, G], fp32)
    w_t = sb("w", [P, G], fp32)
    lssum_t = sb("lssum", [P, 1], fp32)
    draw_t = sb("draw", [P, G], fp32)
    dots_t = sb("dots", [P, G], fp32)
    sim_t = sb("sim", [P, G], fp32)
    rterm_t = sb("rterm", [P, G], fp32)
    rwsum_t = sb("rwsum", [P, 1], fp32)
    onesb_t = sb("onesb", [1, P], bf16)
    sbf1_t = sb("sbf1", [1, D], bf16)
    sbc_t = sb("sbc", [P, D], bf16)
    outsb_t = sb("outsb", [P, 2], fp32)
    warm_t = sb("warm", [1, 2], fp32)

    ctx = ExitStack()
    ps_s = ctx.enter_context(nc.psum_tensor([1, D], fp32))
    ps_bc = ctx.enter_context(nc.psum_tensor([P, D], fp32))
    # single cumulative sems: S_dx counts 16 per landed chunk (one DMA
    # queue completes in order); S_y0 counts chunks whose rsqrt is done
    names = ["S_dx", "S_y0", "S_dsc", "S_ln", "S_sqa", "S_rnbf", "S_pe",
             "S_sbf", "S_pebc", "S_sbc", "S_dve", "S_ones", "S_od"]
    S = {n: ctx.enter_context(nc.semaphore(n)) for n in names}

    def gsl(k):
        return slice(OFF[k], OFF[k] + CH[k])

    with ctx, nc.Block() as block:
        @block.sync
        def _(sync):
            for k in range(NCH):
                sync.dma_start(
                    out=x_t[:, gsl(k), :],
                    in_=xd[:, OFF[k] * D:(OFF[k] + CH[k]) * D],
                ).then_inc(S["S_dx"], 16)
            sync.wait_ge(S["S_dve"], 1)
            sync.dma_start(out=out_d[:], in_=outsb_t[:]).then_inc(S["S_od"], 16)
            sync.wait_ge(S["S_od"], 16)

        @block.gpsimd
        def _(gp):
            gp.dma_start(out=sc_t[:], in_=scd[:]).then_inc(S["S_dsc"], 16)
            # cast the bit-trick rsqrt to bf16 PE weights, per chunk
            for k in range(NCH):
                gp.wait_ge(S["S_y0"], k + 1)
                gp.tensor_copy(out=rnbf_t[:, gsl(k)], in_=y0_t[:, gsl(k)]
                               ).then_inc(S["S_rnbf"], 1)

        @block.scalar
        def _(act):
            # warm Ln first: the natural_log table set also holds
            # square/identity/copy, so this is the only table load
            act.activation(out=warm_t[:, 0:1],
                           in_=nc.const_aps.tensor(1.0, (1, 1)), func=AF.Ln)
            act.activation(out=warm_t[:, 1:2],
                           in_=nc.const_aps.tensor(1.0, (1, 1)), func=AF.Square)
            for k in range(NCH):
                na = ACT_SQ[k]
                if na:
                    act.wait_ge(S["S_dx"], 16 * (k + 1))
                    mm = None
                    for j in range(na):
                        g = OFF[k] + CH[k] - na + j
                        mm = act.activation(
                            out=sqa_t[:, 2 * k + j, :], in_=x_t[:, g, :],
                            func=AF.Square,
                            accum_out=ssq_t[:, g:g + 1],
                        )
                    mm.then_inc(S["S_sqa"], 1)
                if k == 0:
                    # scores logs ride behind chunk 0's work
                    act.wait_ge(S["S_dsc"], 16)
                    act.activation(out=ls_t[:], in_=sc_t[:], func=AF.Ln)
                    act.activation(out=l1_t[:], in_=sc_t[:], func=AF.Ln,
                                   scale=-1.0, bias=1.0).then_inc(S["S_ln"], 1)
            act.wait_ge(S["S_pe"], 1)
            act.copy(sbf1_t[:], ps_s[:]).then_inc(S["S_sbf"], 1)
            act.wait_ge(S["S_pebc"], 1)
            act.copy(sbc_t[:], ps_bc[:]).then_inc(S["S_sbc"], 1)

        @block.vector
        def _(dve):
            dve.memset(onesb_t[:], 1.0)
            dve.memset(outsb_t[:], 0.0).then_inc(S["S_ones"], 1)
            nsq = 0
            for k in range(NCH):
                dve.wait_ge(S["S_dx"], 16 * (k + 1))
                for g in range(OFF[k], OFF[k] + CH[k] - ACT_SQ[k]):
                    dve.scalar_tensor_tensor(
                        out=pt_t[:, g, :], in0=x_t[:, g, :], scalar=0.0,
                        in1=x_t[:, g, :], op0=ALU.bypass, op1=ALU.mult,
                        accum_out=ssq_t[:, g:g + 1],
                    )
                if ACT_SQ[k]:
                    nsq += 1
                    dve.wait_ge(S["S_sqa"], nsq)
                dve.drain()
                # rn for this chunk: rsqrt bit trick on the fp32 ssq
                cs = gsl(k)
                dve.tensor_scalar(
                    out=t1_t[:, cs].bitcast(i32), in0=ssq_t[:, cs].bitcast(i32),
                    scalar1=1, scalar2=-1,
                    op0=ALU.logical_shift_right, op1=ALU.bitwise_xor,
                )
                dve.drain()
                dve.tensor_scalar(
                    out=y0_t[:, cs].bitcast(i32), in0=t1_t[:, cs].bitcast(i32),
                    scalar1=MAGIC + 1, scalar2=None, op0=ALU.add,
                ).then_inc(S["S_y0"], 1)
                if k == 1:
                    # scores tail rides behind chunk 1 (S_ln long since up)
                    dve.wait_ge(S["S_ln"], 1)
                    dve.tensor_scalar_max(ls_t[:], ls_t[:], LOG_CLAMP)
                    dve.tensor_scalar_max(l1_t[:], l1_t[:], LOG_CLAMP)
                    dve.drain()
                    dve.tensor_sub(w_t[:], ls_t[:], l1_t[:])
                    dve.tensor_reduce(out=lssum_t[:], in_=ls_t[:], axis=AX.X,
                                      op=ALU.add)
            # ||s||^2 from the bf16 SBUF copy of s (error ~1e-9 on the loss)
            dve.wait_ge(S["S_sbf"], 1)
            dve.scalar_tensor_tensor(
                out=pt_t[0:1, 0, :], in0=sbf1_t[:], scalar=0.0,
                in1=sbf1_t[:], op0=ALU.bypass, op1=ALU.mult,
                accum_out=outsb_t[0:1, 1:2],
            )
            dve.drain()
            # pass2: per-row dot with broadcast s
            dve.wait_ge(S["S_sbc"], 1)
            for g in range(G):
                dve.scalar_tensor_tensor(
                    out=pt_t[:, g, :], in0=x_t[:, g, :], scalar=0.0,
                    in1=sbc_t[:], op0=ALU.bypass, op1=ALU.mult,
                    accum_out=draw_t[:, g:g + 1],
                )
            dve.drain()
            dve.tensor_mul(dots_t[:], draw_t[:], y0_t[:])
            dve.drain()
            dve.tensor_scalar(
                out=sim_t[:], in0=dots_t[:], scalar1=1.0, scalar2=NINV,
                op0=ALU.subtract, op1=ALU.mult,
            )
            dve.drain()
            dve.scalar_tensor_tensor(
                out=rterm_t[:], in0=sim_t[:], scalar=0.0, in1=w_t[:],
                op0=ALU.max, op1=ALU.mult, accum_out=rwsum_t[:],
            )
            dve.drain()
            dve.tensor_sub(outsb_t[:, 0:1], lssum_t[:], rwsum_t[:]
                           ).then_inc(S["S_dve"], 1)

        @block.tensor
        def _(pe):
            # keep PE busy from the start so HAM unthrottles before the
            # real accumulation matmuls
            pe.wait_ge(S["S_ones"], 1)
            for _ in range(N_WARM):
                pe.matmul(ps_bc[:, 0:P], onesb_t[:], onesb_t[:],
                          start=True, stop=True)
            mm = None
            for k in range(NCH):
                pe.wait_ge(S["S_rnbf"], k + 1)
                for g in range(OFF[k], OFF[k] + CH[k]):
                    mm = pe.matmul(
                        ps_s[:], rnbf_t[:, g:g + 1], x_t[:, g, :],
                        start=(g == 0), stop=(g == G - 1),
                    )
            mm.then_inc(S["S_pe"], 1)
            pe.wait_ge(S["S_sbf"], 1)
            pe.matmul(ps_bc[:], onesb_t[:], sbf1_t[:], start=True, stop=True
                      ).then_inc(S["S_pebc"], 1)

    nc.finalize()
    return nc


def _get_nc():
    if "nc" not in _cache:
        _cache["nc"] = _build_nc()
    return _cache["nc"]


def run_on_device(features: np.ndarray, scores: np.ndarray, trace: bool = False,
                  tmpdir: str | None = None):
    """Returns (per_core_outputs [8, 2] float64, BassKernelResults)."""
    from concourse.bass_utils import run_bass_kernel_spmd

    nc = _get_nc()
    in_maps = []
    for c in range(B):
        in_maps.append({
            "xbf": np.ascontiguousarray(features[c]).reshape(P, G * D)
            .astype(ml_dtypes.bfloat16),
            "scores": np.ascontiguousarray(scores[c]).reshape(P, G)
            .astype(np.float32),
        })
    res = run_bass_kernel_spmd(nc, in_maps, core_ids=list(range(B)),
                               trace=trace, tmpdir=tmpdir)
    # [B, P, 2] per-partition partials; host sums over partitions
    outs = np.stack([res.results[c]["out"].reshape(P, 2).sum(axis=0)
                     for c in range(B)])
    return outs.astype(np.float64), res


def combine(outs: np.ndarray) -> np.float32:
    """Host-side reduction of the 8 per-core [bce_sum, ssq_s] pairs."""
    bce = np.mean(-outs[:, 0] / N)
    feat = 1.0 - np.sum(outs[:, 1]) / (B * float(N) * float(N))
    return np.float32(bce + feat)


def kernel(features: np.ndarray, scores: np.ndarray) -> np.ndarray:
    outs, _ = run_on_device(features, scores)
    return np.asarray(combine(outs), dtype=np.float32)
